# revision 15
# baseline (speedup 1.0000x reference)
"""Trainium2 Bass kernel for nn_Net_4200478015619 (dense_mlp).

Computes, for x (262144, 128) fp32 and W (100, 128) fp32:
    z   = x @ W.T                        # (B, 100)
    y   = z**3 + 0.1 * z
    out = sum(y, axis=1, keepdims=True)  # (B, 1)

Sharding: pure data parallel over 8 NeuronCores - core c gets rows
[c*32768, (c+1)*32768). Each shard is transposed (and cast to bf16) on
the host to xT (128, 32768) so the feature dim lands on SBUF partitions.

Per-core dataflow (64 chunks of 512 batch columns, 22 z tiles of <=3 chunks):
  DMA   : ALL 18 xT tiles issued up-front on the sync queue - the input
          stream free-runs at full HBM rate, never gated by compute.
  PE MM1: z triple [128, 1536] PSUM fp32 = Wpad.T @ x-chunk x3
          (W zero-padded to 128 cols so FWL fast-weight-load engages)
  cube  : y = (z^2 + 0.1) * z in bf16, split by z-triple:
            D  (15): one fused custom DVE op straight from PSUM
            GA' (7): ACT zb=copy(z) [frees PSUM], ACT s=zb^2; Pool y3=s*zb
                     (pure cube; the missing 0.1*z is added by extra PE
                     matmuls contracting zb against alpha-scaled selectors)
  PE MM2: per round r (4 chunks), 4 column-tiled CONCURRENT matmuls
          (tile_position=(0,32g), measured ~3ns stagger) with one-hot
          selector [100,16] stationaries accumulate chunk-sums into rows
          32g+r of ONE persistent [128, 512] PSUM bank; G-chunks add
          alpha*sum(z) via a second accumulating matmul on zb.
  out   : ACT copy [128,512] PSUM->SBUF fp32 + one DMA out.

Host reassembles: out row for chunk j lives in psum row 32*(j%4) + j//4.
"""

import numpy as np

import concourse.bacc as bacc
import concourse.mybir as mybir
import concourse.tile as tile
from concourse.alu_op_type import AluOpType
from concourse.bass_utils import run_bass_kernel_spmd

# --- TileContext exit-drain legalization -----------------------------------
# This toolchain's walrus caps CTRL-class instructions at ONE sync wait; the
# stock TileContext exit drain carries one wait per logical proc and fails
# codegen. Split the waits across per-engine single-wait NOPs instead.
from concourse.vector_clock import ScopedClock, VectorClock


def _patched_drain_and_barrier(self, tick_clock, wait_clock):
    g = tick_clock.global_clock
    n = len(g)
    pending = [i for i in range(n) if g[i] > 0]
    engines = [e for e in self.nc.engines.values()]
    for k, p in enumerate(pending):
        vec = [0] * n
        vec[p] = g[p]
        eng = engines[k % len(engines)]
        nop_inst = eng.nop()
        wait_clock.add_sem_waits(nop_inst.ins, ScopedClock({None: VectorClock(vec)}))
    self.nc.sync.drain()
    self.nc.all_engine_barrier()
    assert self.sems is not None
    popped = self.nc._tile_sem_poison_stack.pop()
    assert popped is self._sem_poison
    self.nc.clear_and_free_semaphores(list(self.sems.allocated().values()))
    self.nc.all_engine_barrier()


tile.TileContext._drain_and_barrier = _patched_drain_and_barrier
# ---------------------------------------------------------------------------


N_CORES = 8
B = 262144
B_CORE = B // N_CORES  # 32768
F = 128
M = 100
MPAD = 128           # W padded to 128 output cols -> FWL weight loads
ALPHA = 0.1
CHUNK = 512          # matmul moving-dim tile (one PSUM bank of fp32)
NCH = B_CORE // CHUNK   # 64 chunks
NROUND = NCH // 4       # 16 MM2 rounds (4 col-tiled chunks each)
TRIPLE = 3           # chunks per z tile
N_WU = 4             # PE warmup matmuls (hold clock ramp through DMA fill)
LAG = 64             # MM2 fully phased to the end (y/zb all buffered)

# Cube-path assignment per z-triple, tuned from HW traces: DVE custom op
# ~1.09ns/elem from PSUM; ACT ~0.96ns/elem; Pool tensor_tensor ~2ns/elem
# (firmware) - usable only spaced out; Pool tensor_scalar ~14ns/elem (dead).
_GA_SET = frozenset((1, 4, 7, 10, 13, 16, 19))
_TRIPLE_PATH = ["GA" if i in _GA_SET else "D" for i in range(21)] + ["D"]

_CUBE_OP = None


def _register_cube_op():
    """Register `out = (Src0^2 + c0) * Src0` as a custom DVE op so the whole
    cubic runs as one Vector instruction straight out of PSUM."""
    global _CUBE_OP
    if _CUBE_OP is not None:
        return _CUBE_OP
    import concourse.dve_ops as dve_ops
    from concourse.dve_spec import Spec, Src0, C0, sq, lower
    from concourse.dve_uop import DveOpSpec

    name = "CUBE_AXPB_ANT"
    for op in dve_ops.OPS:
        if op.name == name:
            _CUBE_OP = op
            return op
    spec = Spec(
        body=(sq(Src0) + C0) * Src0,
        reference=lambda in0, in1, s0, s1, imm2: (
            (in0.astype(np.float32) ** 2 + s0) * in0.astype(np.float32)
        ).astype(np.float32),
    )
    row = dve_ops._CUSTOM_DVE_ROW_BASE + len(dve_ops.OPS)
    assert row < 0x20, "custom-DVE opcode rows exhausted"
    shas = {
        ver: DveOpSpec(
            name=name, opcode=row, uops=lower(spec, ver=ver), rd1_en=False
        ).sha(ver)
        for ver in ("v3", "v4")
    }
    op = dve_ops.DveOp(name, spec, subdim=False, uops_sha=shas)
    dve_ops.OPS.append(op)
    dve_ops._SUB_OPCODE_FOR_NAME[name] = row
    dve_ops.CUSTOM_DVE_SPECS[name] = spec
    _CUBE_OP = op
    return op


def build_nc():
    cube_op = _register_cube_op()
    nc = bacc.Bacc()
    xt = nc.declare_dram_parameter("xt", [F, B_CORE], mybir.dt.bfloat16, isOutput=False)
    wt = nc.declare_dram_parameter("wt", [F, MPAD], mybir.dt.bfloat16, isOutput=False)
    # sel block r ([100, 16]): column r all-ones; sela blocks are the same
    # scaled by ALPHA (adds alpha*sum_m z for G-path chunks).
    sel = nc.declare_dram_parameter(
        "sel", [M, 16 * NROUND], mybir.dt.bfloat16, isOutput=False
    )
    sela = nc.declare_dram_parameter(
        "sela", [M, 16 * NROUND], mybir.dt.bfloat16, isOutput=False
    )
    out = nc.declare_dram_parameter(
        "out", [MPAD, CHUNK], mybir.dt.float32, isOutput=True
    )
    junk = nc.declare_dram_parameter("junk", [1, 8], mybir.dt.float32, isOutput=True)

    with tile.TileContext(nc) as tc:
        with (
            tc.tile_pool(name="wpool", bufs=1) as wpool,
            tc.tile_pool(name="xpool", bufs=18) as xpool,
            tc.tile_pool(name="spool", bufs=2) as spool,
            tc.tile_pool(name="zbpool", bufs=7) as zbpool,
            tc.tile_pool(name="ypool", bufs=22) as ypool,
            tc.tile_pool(name="opool", bufs=1) as opool,
            tc.tile_pool(name="zpsum", bufs=2, space="PSUM") as zpsum,
            tc.tile_pool(name="opsum", bufs=1, space="PSUM") as opsum,
        ):
            # --- t=0 warmups (overlap the DMA fill) ---
            wu_w = wpool.tile([F, MPAD], mybir.dt.bfloat16)
            nc.gpsimd.memset(wu_w[:], 0.0)
            wu_x = wpool.tile([F, CHUNK], mybir.dt.bfloat16)
            nc.gpsimd.memset(wu_x[:], 0.0)
            # ACT warmup: loads the Copy/Square activation table early.
            wu_s = wpool.tile([F, 128], mybir.dt.bfloat16)
            nc.scalar.square(out=wu_s[:], in_=wu_x[:, :128])
            # PE warmup; the junk output makes it un-eliminable.
            wu_p = zpsum.tile([MPAD, TRIPLE * CHUNK], mybir.dt.float32, tag="z")
            for i in range(N_WU):
                nc.tensor.matmul(
                    wu_p[:, (i % TRIPLE) * CHUNK : (i % TRIPLE + 1) * CHUNK],
                    lhsT=wu_w[:],
                    rhs=wu_x[:],
                    start=True,
                    stop=True,
                )
            wu_j = wpool.tile([1, 8], mybir.dt.float32)
            nc.vector.tensor_copy(out=wu_j[:], in_=wu_p[0:1, 0:8])
            nc.gpsimd.dma_start(out=junk[:], in_=wu_j[:])

            # params ride the Pool HWDGE queue; x stream owns the sync queue
            ws = wpool.tile([F, MPAD], mybir.dt.bfloat16)
            nc.gpsimd.dma_start(out=ws[:], in_=wt[:])
            sel_s = wpool.tile([M, 16 * NROUND], mybir.dt.bfloat16)
            nc.gpsimd.dma_start(out=sel_s[:], in_=sel[:])
            sela_s = wpool.tile([M, 16 * NROUND], mybir.dt.bfloat16)
            nc.gpsimd.dma_start(out=sela_s[:], in_=sela[:])

            # persistent output accumulator: one PSUM bank, 64 chunk-sums
            o_acc = opsum.tile([MPAD, CHUNK], mybir.dt.float32)

            # front-loaded input stream: every tile DMA issued immediately
            widths = [512, 1024] + [2048] * 15 + [512]
            assert sum(widths) == B_CORE
            xtiles = []
            col = 0
            for width in widths:
                xs = xpool.tile([F, width], mybir.dt.bfloat16)
                nc.sync.dma_start(out=xs[:], in_=xt[:, col : col + width])
                xtiles.append((xs, col, width))
                col += width

            ydeck = {}   # triple -> y tile (bf16 [100, <=1536])
            zbdeck = {}  # triple -> zb tile for G-path (alpha matmuls)
            cubes_done = 0
            rounds_done = 0
            stage_b = []  # deferred SBUF-side cube work

            def emit_cube(tri, zt, w):
                zin = zt[0:M, :w]
                y = ypool.tile([M, TRIPLE * CHUNK], mybir.dt.bfloat16, tag="y")
                if _TRIPLE_PATH[tri] == "D":
                    nc.vector._custom_dve(cube_op, out=y[:, :w], in0=zin, s0=ALPHA)
                else:  # GA': pure cube; alpha comes from the PE side
                    zb = zbpool.tile([M, TRIPLE * CHUNK], mybir.dt.bfloat16, tag="zb")
                    nc.scalar.copy(out=zb[:, :w], in_=zin)
                    zbdeck[tri] = zb

                    def rest(zb=zb, y=y, w=w):
                        s = spool.tile([M, TRIPLE * CHUNK], mybir.dt.bfloat16, tag="s")
                        nc.scalar.square(out=s[:, :w], in_=zb[:, :w])
                        nc.gpsimd.tensor_tensor(
                            out=y[:, :w], in0=s[:, :w], in1=zb[:, :w],
                            op=AluOpType.mult,
                        )

                    stage_b.append(rest)
                ydeck[tri] = y
                while len(stage_b) > 1:
                    stage_b.pop(0)()

            def flush_rounds():
                nonlocal rounds_done
                while rounds_done < NROUND and cubes_done >= min(
                    NCH, 4 * (rounds_done + 1) + LAG
                ):
                    r = rounds_done
                    for g in range(4):
                        j = 4 * r + g
                        y = ydeck[j // TRIPLE]
                        c = j % TRIPLE
                        nc.tensor.matmul(
                            o_acc[32 * g : 32 * g + 16, :],
                            lhsT=sel_s[:, 16 * r : 16 * r + 16],
                            rhs=y[:, c * CHUNK : (c + 1) * CHUNK],
                            start=(r == 0),
                            stop=(r == NROUND - 1),
                            tile_position=(0, 32 * g),
                        )
                    for g in range(4):
                        j = 4 * r + g
                        tri = j // TRIPLE
                        if _TRIPLE_PATH[tri] != "D":
                            zb = zbdeck[tri]
                            c = j % TRIPLE
                            nc.tensor.matmul(
                                o_acc[32 * g : 32 * g + 16, :],
                                lhsT=sela_s[:, 16 * r : 16 * r + 16],
                                rhs=zb[:, c * CHUNK : (c + 1) * CHUNK],
                                start=False,
                                stop=False,
                                tile_position=(0, 32 * g),
                            )
                    rounds_done += 1
                    last_needed = (4 * rounds_done) // TRIPLE
                    for t in [t for t in ydeck if t < last_needed]:
                        del ydeck[t]
                        zbdeck.pop(t, None)

            chunk = 0
            zt = None
            for xs, col, width in xtiles:
                for lc in range(width // CHUNK):
                    tri, off = chunk // TRIPLE, chunk % TRIPLE
                    if off == 0:
                        zt = zpsum.tile(
                            [MPAD, TRIPLE * CHUNK], mybir.dt.float32, tag="z"
                        )
                    nc.tensor.matmul(
                        zt[:, off * CHUNK : (off + 1) * CHUNK],
                        lhsT=ws[:],
                        rhs=xs[:, lc * CHUNK : (lc + 1) * CHUNK],
                        start=True,
                        stop=True,
                    )
                    chunk += 1
                    if off == TRIPLE - 1 or chunk == NCH:
                        emit_cube(tri, zt, (off + 1) * CHUNK)
                        cubes_done = chunk
                        if chunk == NCH:
                            while stage_b:
                                stage_b.pop(0)()
                        flush_rounds()
            assert rounds_done == NROUND, rounds_done

            osb = opool.tile([MPAD, CHUNK], mybir.dt.float32)
            nc.scalar.copy(out=osb[:], in_=o_acc[:])
            nc.sync.dma_start(out=out[:], in_=osb[:])
    nc.finalize()
    return nc


def _run(x, W, trace=False, **run_kwargs):
    import ml_dtypes

    x = np.ascontiguousarray(x, dtype=np.float32)
    W = np.ascontiguousarray(W, dtype=np.float32)
    wt_np = np.zeros((F, MPAD), dtype=ml_dtypes.bfloat16)
    wt_np[:, :M] = W.T.astype(ml_dtypes.bfloat16)  # (128, 100) padded to 128

    sel_np = np.zeros((M, 16 * NROUND), dtype=np.float32)
    for r in range(NROUND):
        sel_np[:, 16 * r + r] = 1.0
    sela_np = (sel_np * ALPHA).astype(ml_dtypes.bfloat16)
    sel_np = sel_np.astype(ml_dtypes.bfloat16)

    in_maps = []
    for c in range(N_CORES):
        shard = x[c * B_CORE : (c + 1) * B_CORE, :]  # (32768, 128)
        xt_np = np.ascontiguousarray(shard.T.astype(ml_dtypes.bfloat16))
        in_maps.append({"xt": xt_np, "wt": wt_np, "sel": sel_np, "sela": sela_np})

    nc = build_nc()
    res = run_bass_kernel_spmd(
        nc, in_maps, list(range(N_CORES)), trace=trace, **run_kwargs
    )
    outs = []
    for c in range(N_CORES):
        r = res.results[c]["out"]  # [128, 512]; chunk j -> row 32*(j%4)+j//4
        full = r.reshape(4, 32, CHUNK)[:, :NROUND, :]  # [g, r, col]
        outs.append(np.ascontiguousarray(full.transpose(1, 0, 2)).reshape(B_CORE, 1))
    return np.concatenate(outs, axis=0), res


def kernel(x, W):
    full, _ = _run(x, W)
    return full


# revision 16
# speedup vs baseline: 1.2609x; 1.2609x over previous
"""Trainium2 Bass kernel for nn_Net_4200478015619 (dense_mlp).

Computes, for x (262144, 128) fp32 and W (100, 128) fp32:
    z   = x @ W.T                        # (B, 100)
    y   = z**3 + 0.1 * z
    out = sum(y, axis=1, keepdims=True)  # (B, 1)

Sharding: pure data parallel over 8 NeuronCores - core c gets rows
[c*32768, (c+1)*32768). Each shard is transposed (and cast to bf16) on
the host to xT (128, 32768) so the feature dim lands on SBUF partitions.

Per-core dataflow (64 chunks of 512 batch columns, 22 z tiles of <=3 chunks):
  DMA   : ALL 18 xT tiles issued up-front on the sync queue - the input
          stream free-runs at full HBM rate, never gated by compute.
  PE MM1: z triple [128, 1536] PSUM fp32 = Wpad.T @ x-chunk x3
          (W zero-padded to 128 cols so FWL fast-weight-load engages)
  cube  : y = (z^2 + 0.1) * z in bf16, split by z-triple:
            D  (15): one fused custom DVE op straight from PSUM
            GA' (7): ACT zb=copy(z) [frees PSUM], ACT s=zb^2; Pool y3=s*zb
                     (pure cube; the missing 0.1*z is added by extra PE
                     matmuls contracting zb against alpha-scaled selectors)
  PE MM2: per round r (4 chunks), 4 column-tiled CONCURRENT matmuls
          (tile_position=(0,32g), measured ~3ns stagger) with one-hot
          selector [100,16] stationaries accumulate chunk-sums into rows
          32g+r of ONE persistent [128, 512] PSUM bank; G-chunks add
          alpha*sum(z) via a second accumulating matmul on zb.
  out   : ACT copy [128,512] PSUM->SBUF fp32 + one DMA out.

Host reassembles: out row for chunk j lives in psum row 32*(j%4) + j//4.
"""

import numpy as np

import concourse.bacc as bacc
import concourse.mybir as mybir
import concourse.tile as tile
from concourse.alu_op_type import AluOpType
from concourse.bass_utils import run_bass_kernel_spmd

# --- TileContext exit-drain legalization -----------------------------------
# This toolchain's walrus caps CTRL-class instructions at ONE sync wait; the
# stock TileContext exit drain carries one wait per logical proc and fails
# codegen. Split the waits across per-engine single-wait NOPs instead.
from concourse.vector_clock import ScopedClock, VectorClock


def _patched_drain_and_barrier(self, tick_clock, wait_clock):
    g = tick_clock.global_clock
    n = len(g)
    pending = [i for i in range(n) if g[i] > 0]
    engines = [e for e in self.nc.engines.values()]
    for k, p in enumerate(pending):
        vec = [0] * n
        vec[p] = g[p]
        eng = engines[k % len(engines)]
        nop_inst = eng.nop()
        wait_clock.add_sem_waits(nop_inst.ins, ScopedClock({None: VectorClock(vec)}))
    self.nc.sync.drain()
    self.nc.all_engine_barrier()
    assert self.sems is not None
    popped = self.nc._tile_sem_poison_stack.pop()
    assert popped is self._sem_poison
    self.nc.clear_and_free_semaphores(list(self.sems.allocated().values()))
    self.nc.all_engine_barrier()


tile.TileContext._drain_and_barrier = _patched_drain_and_barrier
# ---------------------------------------------------------------------------


N_CORES = 8
B = 262144
B_CORE = B // N_CORES  # 32768
F = 128
M = 100
MPAD = 128           # W padded to 128 output cols -> FWL weight loads
ALPHA = 0.1
CHUNK = 512          # matmul moving-dim tile (one PSUM bank of fp32)
NCH = B_CORE // CHUNK   # 64 chunks
NROUND = NCH // 4       # 16 MM2 rounds (4 col-tiled chunks each)
TRIPLE = 2           # chunks per z tile (pairs: 2 banks x 3 bufs decouple)
N_WU = 4             # PE warmup matmuls (hold clock ramp through DMA fill)
LAG = 64             # MM2 fully phased to the end (y/zb all buffered)

# Cube-path assignment per z-triple, tuned from HW traces: DVE custom op
# ~1.09ns/elem from PSUM; ACT ~0.96ns/elem; Pool tensor_tensor ~2ns/elem
# (firmware) - usable only spaced out; Pool tensor_scalar ~14ns/elem (dead).
_GA_SET = frozenset((2, 5, 8, 11, 14, 17, 20, 23, 26, 29))
_TRIPLE_PATH = ["GA" if i in _GA_SET else "D" for i in range(32)]

_CUBE_OP = None


def _register_cube_op():
    """Register `out = (Src0^2 + c0) * Src0` as a custom DVE op so the whole
    cubic runs as one Vector instruction straight out of PSUM."""
    global _CUBE_OP
    if _CUBE_OP is not None:
        return _CUBE_OP
    import concourse.dve_ops as dve_ops
    from concourse.dve_spec import Spec, Src0, C0, sq, lower
    from concourse.dve_uop import DveOpSpec

    name = "CUBE_AXPB_ANT"
    for op in dve_ops.OPS:
        if op.name == name:
            _CUBE_OP = op
            return op
    spec = Spec(
        body=(sq(Src0) + C0) * Src0,
        reference=lambda in0, in1, s0, s1, imm2: (
            (in0.astype(np.float32) ** 2 + s0) * in0.astype(np.float32)
        ).astype(np.float32),
    )
    row = dve_ops._CUSTOM_DVE_ROW_BASE + len(dve_ops.OPS)
    assert row < 0x20, "custom-DVE opcode rows exhausted"
    shas = {
        ver: DveOpSpec(
            name=name, opcode=row, uops=lower(spec, ver=ver), rd1_en=False
        ).sha(ver)
        for ver in ("v3", "v4")
    }
    op = dve_ops.DveOp(name, spec, subdim=False, uops_sha=shas)
    dve_ops.OPS.append(op)
    dve_ops._SUB_OPCODE_FOR_NAME[name] = row
    dve_ops.CUSTOM_DVE_SPECS[name] = spec
    _CUBE_OP = op
    return op


def build_nc():
    cube_op = _register_cube_op()
    nc = bacc.Bacc()
    xt = nc.declare_dram_parameter("xt", [F, B_CORE], mybir.dt.bfloat16, isOutput=False)
    wt = nc.declare_dram_parameter("wt", [F, MPAD], mybir.dt.bfloat16, isOutput=False)
    # sel block r ([100, 16]): column r all-ones; sela blocks are the same
    # scaled by ALPHA (adds alpha*sum_m z for G-path chunks).
    sel = nc.declare_dram_parameter(
        "sel", [M, 16 * NROUND], mybir.dt.bfloat16, isOutput=False
    )
    sela = nc.declare_dram_parameter(
        "sela", [M, 16 * NROUND], mybir.dt.bfloat16, isOutput=False
    )
    out = nc.declare_dram_parameter(
        "out", [MPAD, CHUNK], mybir.dt.float32, isOutput=True
    )
    junk = nc.declare_dram_parameter("junk", [1, 8], mybir.dt.float32, isOutput=True)

    with tile.TileContext(nc) as tc:
        with (
            tc.tile_pool(name="wpool", bufs=1) as wpool,
            tc.tile_pool(name="xpool", bufs=18) as xpool,
            tc.tile_pool(name="spool", bufs=2) as spool,
            tc.tile_pool(name="zbpool", bufs=10) as zbpool,
            tc.tile_pool(name="ypool", bufs=32) as ypool,
            tc.tile_pool(name="opool", bufs=1) as opool,
            tc.tile_pool(name="zpsum", bufs=3, space="PSUM") as zpsum,
            tc.tile_pool(name="opsum", bufs=1, space="PSUM") as opsum,
        ):
            # --- t=0 warmups (overlap the DMA fill) ---
            wu_w = wpool.tile([F, MPAD], mybir.dt.bfloat16)
            nc.gpsimd.memset(wu_w[:], 0.0)
            wu_x = wpool.tile([F, CHUNK], mybir.dt.bfloat16)
            nc.gpsimd.memset(wu_x[:], 0.0)
            # ACT warmup: loads the Copy/Square activation table early.
            wu_s = wpool.tile([F, 128], mybir.dt.bfloat16)
            nc.scalar.square(out=wu_s[:], in_=wu_x[:, :128])
            # PE warmup; the junk output makes it un-eliminable.
            wu_p = zpsum.tile([MPAD, TRIPLE * CHUNK], mybir.dt.float32, tag="z")
            for i in range(N_WU):
                nc.tensor.matmul(
                    wu_p[:, (i % TRIPLE) * CHUNK : (i % TRIPLE + 1) * CHUNK],
                    lhsT=wu_w[:],
                    rhs=wu_x[:],
                    start=True,
                    stop=True,
                )
            wu_j = wpool.tile([1, 8], mybir.dt.float32)
            nc.vector.tensor_copy(out=wu_j[:], in_=wu_p[0:1, 0:8])
            nc.gpsimd.dma_start(out=junk[:], in_=wu_j[:])

            # params ride the Pool HWDGE queue; x stream owns the sync queue
            ws = wpool.tile([F, MPAD], mybir.dt.bfloat16)
            nc.gpsimd.dma_start(out=ws[:], in_=wt[:])
            sel_s = wpool.tile([M, 16 * NROUND], mybir.dt.bfloat16)
            nc.gpsimd.dma_start(out=sel_s[:], in_=sel[:])
            sela_s = wpool.tile([M, 16 * NROUND], mybir.dt.bfloat16)
            nc.gpsimd.dma_start(out=sela_s[:], in_=sela[:])

            # persistent output accumulator: one PSUM bank, 64 chunk-sums
            o_acc = opsum.tile([MPAD, CHUNK], mybir.dt.float32)

            # front-loaded input stream: every tile DMA issued immediately
            widths = [512, 1024] + [2048] * 15 + [512]
            assert sum(widths) == B_CORE
            xtiles = []
            col = 0
            for width in widths:
                xs = xpool.tile([F, width], mybir.dt.bfloat16)
                nc.sync.dma_start(out=xs[:], in_=xt[:, col : col + width])
                xtiles.append((xs, col, width))
                col += width

            ydeck = {}   # triple -> y tile (bf16 [100, <=1536])
            zbdeck = {}  # triple -> zb tile for G-path (alpha matmuls)
            cubes_done = 0
            rounds_done = 0
            stage_b = []  # deferred SBUF-side cube work

            def emit_cube(tri, zt, w):
                zin = zt[0:M, :w]
                y = ypool.tile([M, TRIPLE * CHUNK], mybir.dt.bfloat16, tag="y")
                if _TRIPLE_PATH[tri] == "D":
                    nc.vector._custom_dve(cube_op, out=y[:, :w], in0=zin, s0=ALPHA)
                else:  # GA': pure cube; alpha comes from the PE side
                    zb = zbpool.tile([M, TRIPLE * CHUNK], mybir.dt.bfloat16, tag="zb")
                    nc.scalar.copy(out=zb[:, :w], in_=zin)
                    zbdeck[tri] = zb

                    def rest(zb=zb, y=y, w=w):
                        s = spool.tile([M, TRIPLE * CHUNK], mybir.dt.bfloat16, tag="s")
                        nc.scalar.square(out=s[:, :w], in_=zb[:, :w])
                        nc.gpsimd.tensor_tensor(
                            out=y[:, :w], in0=s[:, :w], in1=zb[:, :w],
                            op=AluOpType.mult,
                        )

                    stage_b.append(rest)
                ydeck[tri] = y
                while len(stage_b) > 1:
                    stage_b.pop(0)()

            def flush_rounds():
                nonlocal rounds_done
                while rounds_done < NROUND and cubes_done >= min(
                    NCH, 4 * (rounds_done + 1) + LAG
                ):
                    r = rounds_done
                    for g in range(4):
                        j = 4 * r + g
                        y = ydeck[j // TRIPLE]
                        c = j % TRIPLE
                        nc.tensor.matmul(
                            o_acc[32 * g : 32 * g + 16, :],
                            lhsT=sel_s[:, 16 * r : 16 * r + 16],
                            rhs=y[:, c * CHUNK : (c + 1) * CHUNK],
                            start=(r == 0),
                            stop=(r == NROUND - 1),
                            tile_position=(0, 32 * g),
                        )
                    for g in range(4):
                        j = 4 * r + g
                        tri = j // TRIPLE
                        if _TRIPLE_PATH[tri] != "D":
                            zb = zbdeck[tri]
                            c = j % TRIPLE
                            nc.tensor.matmul(
                                o_acc[32 * g : 32 * g + 16, :],
                                lhsT=sela_s[:, 16 * r : 16 * r + 16],
                                rhs=zb[:, c * CHUNK : (c + 1) * CHUNK],
                                start=False,
                                stop=False,
                                tile_position=(0, 32 * g),
                            )
                    rounds_done += 1
                    last_needed = (4 * rounds_done) // TRIPLE
                    for t in [t for t in ydeck if t < last_needed]:
                        del ydeck[t]
                        zbdeck.pop(t, None)

            chunk = 0
            zt = None
            for xs, col, width in xtiles:
                for lc in range(width // CHUNK):
                    tri, off = chunk // TRIPLE, chunk % TRIPLE
                    if off == 0:
                        nc.tensor.matmul(
                            o_acc[:], lhsT=ws[:], rhs=wu_x[:],
                            start=True, stop=True,
                        )
                        zt = zpsum.tile(
                            [MPAD, TRIPLE * CHUNK], mybir.dt.float32, tag="z"
                        )
                    nc.tensor.matmul(
                        zt[:, off * CHUNK : (off + 1) * CHUNK],
                        lhsT=ws[:],
                        rhs=xs[:, lc * CHUNK : (lc + 1) * CHUNK],
                        start=True,
                        stop=True,
                    )
                    chunk += 1
                    if off == TRIPLE - 1 or chunk == NCH:
                        emit_cube(tri, zt, (off + 1) * CHUNK)
                        cubes_done = chunk
                        if chunk == NCH:
                            while stage_b:
                                stage_b.pop(0)()
                        flush_rounds()
            assert rounds_done == NROUND, rounds_done

            osb = opool.tile([MPAD, CHUNK], mybir.dt.float32)
            nc.scalar.copy(out=osb[:], in_=o_acc[:])
            nc.sync.dma_start(out=out[:], in_=osb[:])
    nc.finalize()
    return nc


def _run(x, W, trace=False, **run_kwargs):
    import ml_dtypes

    x = np.ascontiguousarray(x, dtype=np.float32)
    W = np.ascontiguousarray(W, dtype=np.float32)
    wt_np = np.zeros((F, MPAD), dtype=ml_dtypes.bfloat16)
    wt_np[:, :M] = W.T.astype(ml_dtypes.bfloat16)  # (128, 100) padded to 128

    sel_np = np.zeros((M, 16 * NROUND), dtype=np.float32)
    for r in range(NROUND):
        sel_np[:, 16 * r + r] = 1.0
    sela_np = (sel_np * ALPHA).astype(ml_dtypes.bfloat16)
    sel_np = sel_np.astype(ml_dtypes.bfloat16)

    in_maps = []
    for c in range(N_CORES):
        shard = x[c * B_CORE : (c + 1) * B_CORE, :]  # (32768, 128)
        xt_np = np.ascontiguousarray(shard.T.astype(ml_dtypes.bfloat16))
        in_maps.append({"xt": xt_np, "wt": wt_np, "sel": sel_np, "sela": sela_np})

    nc = build_nc()
    res = run_bass_kernel_spmd(
        nc, in_maps, list(range(N_CORES)), trace=trace, **run_kwargs
    )
    outs = []
    for c in range(N_CORES):
        r = res.results[c]["out"]  # [128, 512]; chunk j -> row 32*(j%4)+j//4
        full = r.reshape(4, 32, CHUNK)[:, :NROUND, :]  # [g, r, col]
        outs.append(np.ascontiguousarray(full.transpose(1, 0, 2)).reshape(B_CORE, 1))
    return np.concatenate(outs, axis=0), res


def kernel(x, W):
    full, _ = _run(x, W)
    return full
